# revision 4
# baseline (speedup 1.0000x reference)
"""Trainium2 Bass kernel v2 for nn_Decoder (GRU decoder + vocab projection).

Changes vs baseline:
  - scan chain: one SIG op (z,r) on ACT, n-strip evicted on DVE in parallel;
    H-layout gate math reads the transpose PSUM directly (no gateT copy);
    update h = (1-z)n + z h with zp/zh off the critical chain.
  - x128 scaling: W_hh, gx, bn scaled x128 host-side; SIG/TANH use scale=1/128.
    This lets the gx pipeline run in fp8 (gate preacts are dominated by the
    h@W_hh term, so fp8 noise on gx is negligible).
  - gx matmuls: fp8e4 DoubleRow (K=256 per instruction), full-array M=128.
  - pred flipped: out[vocab, B*S]; bias is a per-partition scalar fused into
    the DVE eviction; DMA straight from SBUF stage; host transposes at the
    end (free). Staggered bt-chunks keep the post-scan tail small.
"""
import numpy as np
import ml_dtypes
from contextlib import ExitStack

import concourse.bass as bass
import concourse.tile as tile
from concourse import bacc, mybir
from concourse import bass_utils
from concourse.masks import make_identity

f32 = mybir.dt.float32
f32r = mybir.dt.float32r
bf16 = mybir.dt.bfloat16
fp8 = mybir.dt.float8e4
i32 = mybir.dt.int32
SIG = mybir.ActivationFunctionType.Sigmoid
TANH = mybir.ActivationFunctionType.Tanh
MUL = mybir.AluOpType.mult
ADD = mybir.AluOpType.add
DR = mybir.MatmulPerfMode.DoubleRow

V, E, EH, H = 32000, 512, 1024, 512
B, S = 16, 128
NC = 8
VS = V // NC          # 4000 vocab rows per core
NT = (B * S) // 128   # 16 (t-major) tiles of 128 bt rows
G3 = 3 * H            # 1536
GSC = 128.0           # gate pre-activation scale

# pred bt-chunks: (bt_offset, n_cols); chunk ready after step (bto+n)/16
PRED_CHUNKS = [(0, 512), (512, 512), (1024, 512), (1536, 256), (1792, 128), (1920, 128)]

_PROG_CACHE = {}


def build_program():
    if "nc" in _PROG_CACHE:
        return _PROG_CACHE["nc"]
    nc = bacc.Bacc("TRN2", target_bir_lowering=False, debug=False,
                   enable_asserts=False, num_devices=NC)

    # ---------------- DRAM I/O ----------------
    EMB = nc.dram_tensor("emb", [V, E], f32, kind="ExternalInput").ap()
    YT = nc.dram_tensor("y_tm", [B * S, 1], i32, kind="ExternalInput").ap()
    HNT = nc.dram_tensor("hnT", [128, 8 * 16], f32r, kind="ExternalInput").ap()
    FCWT = nc.dram_tensor("fcwT", [128, 8 * 512], f32r, kind="ExternalInput").ap()
    FCBT = nc.dram_tensor("fcbT", [128, 4], f32, kind="ExternalInput").ap()
    WHHT = nc.dram_tensor("whhT", [128, 4 * G3], bf16, kind="ExternalInput").ap()
    WIHT8 = nc.dram_tensor("wihT8", [128, 2 * 2 * 3 * 512], fp8, kind="ExternalInput").ap()
    BIASRZ = nc.dram_tensor("bias_rz", [128, 1024], bf16, kind="ExternalInput").ap()
    BIASNT = nc.dram_tensor("bias_nT", [128, 4], f32, kind="ExternalInput").ap()
    BNB = nc.dram_tensor("bnb", [32, 512], bf16, kind="ExternalInput").ap()
    SEL = nc.dram_tensor("sel", [128, 64], bf16, kind="ExternalInput").ap()
    PREDWT = nc.dram_tensor("predwT", [128, 32 * 4 * 128], bf16, kind="ExternalInput").ap()
    PREDBT = nc.dram_tensor("predbT", [128, 32], f32, kind="ExternalInput").ap()
    OUT = nc.dram_tensor("out", [VS, B * S], bf16, kind="ExternalOutput").ap()

    with tile.TileContext(nc) as tc:
        with ExitStack() as ctx:
            pers = ctx.enter_context(tc.tile_pool(name="pers", bufs=1))
            sb2 = ctx.enter_context(tc.tile_pool(name="sb2", bufs=2))
            sb3 = ctx.enter_context(tc.tile_pool(name="sb3", bufs=6))
            psg = ctx.enter_context(tc.tile_pool(name="psg", bufs=2, space="PSUM"))
            pst = ctx.enter_context(tc.tile_pool(name="pst", bufs=2, space="PSUM"))
            psp = ctx.enter_context(tc.tile_pool(name="psp", bufs=3, space="PSUM"))

            # ---------- persistent tiles ----------
            whhT = pers.tile([128, 4 * G3], bf16)
            wihT8 = pers.tile([128, 2 * 2 * 3 * 512], fp8)
            predwT = pers.tile([128, 32 * 4 * 128], bf16)
            predbT = pers.tile([128, 32], f32)
            bias_rz = pers.tile([128, 1024], bf16)
            bias_nT = pers.tile([128, 4], f32)
            bnb = pers.tile([32, 512], bf16)
            sel = pers.tile([128, 64], bf16)
            ident = pers.tile([128, 128], f32)
            ident_bf = pers.tile([128, 128], bf16)
            gx_rz = pers.tile([128, NT * 1024], bf16)
            gxnT = pers.tile([128, 4 * 2048], bf16)
            outT = pers.tile([128, 4 * 2048 + 16], bf16)
            h0bf = pers.tile([128, 80], bf16)
            state = pers.tile([128, 2 * 64], f32)

            nc.sync.dma_start(whhT[:], WHHT)
            nc.sync.dma_start(wihT8[:], WIHT8)
            nc.sync.dma_start(predwT[:], PREDWT)
            nc.sync.dma_start(predbT[:], PREDBT)
            nc.sync.dma_start(bias_rz[:], BIASRZ)
            nc.sync.dma_start(bias_nT[:], BIASNT)
            nc.sync.dma_start(bnb[:], BNB)
            nc.sync.dma_start(sel[:], SEL)
            make_identity(nc, ident[:])
            make_identity(nc, ident_bf[:])
            nc.gpsimd.memset(outT[:], 0.0)
            nc.gpsimd.memset(h0bf[:], 0.0)

            # ---------- fc -> h0 (state slot 1 = "h_{-1}") ----------
            hnT = sb2.tile([128, 8 * 16], f32r, tag="hnT")
            fcwT = pers.tile([128, 8 * 512], f32r)
            fcbT = sb2.tile([128, 4], f32, tag="fcbT")
            nc.sync.dma_start(hnT[:], HNT)
            nc.sync.dma_start(fcwT[:], FCWT)
            nc.sync.dma_start(fcbT[:], FCBT)
            pfc = psp.tile([128, 512], f32, tag="pp")
            for kc in range(8):
                nc.tensor.matmul(
                    out=pfc[0:16, :],
                    lhsT=hnT[:, 16 * kc:16 * kc + 16],
                    rhs=fcwT[:, 512 * kc:512 * kc + 512],
                    start=(kc == 0), stop=(kc == 7),
                )
            fcbuf = sb2.tile([128, 512], f32, tag="fcbuf")
            nc.vector.tensor_copy(fcbuf[0:16, :], pfc[0:16, :])
            p2fc = pst.tile([128, 320], f32, tag="p2")
            for hc in range(4):
                nc.tensor.transpose(
                    out=p2fc[:, 16 * hc:16 * hc + 16],
                    in_=fcbuf[0:16, 128 * hc:128 * hc + 128],
                    identity=ident[0:16, 0:16],
                )
            for hc in range(4):
                nc.vector.tensor_scalar(
                    out=state[:, 64 + 16 * hc:64 + 16 * hc + 16],
                    in0=p2fc[:, 16 * hc:16 * hc + 16],
                    scalar1=fcbT[:, hc:hc + 1], scalar2=None, op0=ADD,
                )
            nc.vector.tensor_copy(h0bf[:, 0:64].rearrange("p (hc c) -> p hc c", hc=4),
                                  state[:, 64:128].rearrange("p (hc c) -> p hc c", hc=4))

            # ---------- gx tile: gather + fp8 DoubleRow input projection ----------
            def emit_gx_tile(j):
                ytile = sb3.tile([128, 1], i32, tag="ytile")
                nc.sync.dma_start(ytile[:], YT[128 * j:128 * j + 128, :])
                xt = sb3.tile([128, 512], f32, tag="xt")
                nc.gpsimd.indirect_dma_start(
                    out=xt[:], out_offset=None, in_=EMB,
                    in_offset=bass.IndirectOffsetOnAxis(ap=ytile[:, :1], axis=0),
                )
                px = psp.tile([128, 512], f32, tag="pp")
                for ec in range(4):
                    nc.tensor.transpose(
                        out=px[:, 128 * ec:128 * ec + 128],
                        in_=xt[:, 128 * ec:128 * ec + 128],
                        identity=ident[:, :],
                    )
                xT8 = sb3.tile([128, 512], fp8, tag="xT8")
                nc.scalar.mul(xT8[:], px[:], 32.0)
                x8v = xT8[:, :].rearrange("p (pair kt bt) -> p pair kt bt", pair=2, kt=2)
                w8v = wihT8[:, :].rearrange("p (pair kt g c) -> p pair kt g c",
                                            pair=2, kt=2, g=3)
                for g in range(3):
                    pgx = psp.tile([128, 512], f32, tag="pp")
                    for pr in range(2):
                        nc.tensor.matmul(
                            out=pgx[:, :],
                            lhsT=x8v[:, pr, :, :],
                            rhs=w8v[:, pr, :, g, :],
                            start=(pr == 0), stop=(pr == 1),
                            perf_mode=DR,
                        )
                    if g < 2:
                        nc.vector.tensor_tensor(
                            out=gx_rz[:, 1024 * j + 512 * g:1024 * j + 512 * g + 512],
                            in0=pgx[:, :], in1=bias_rz[:, 512 * g:512 * g + 512],
                            op=ADD,
                        )
                    else:
                        nstage = sb3.tile([128, 512], bf16, tag="nstage")
                        nc.scalar.copy(nstage[:], pgx[:])
                        p2n = pst.tile([128, 512], bf16, tag="p2")
                        for hc in range(4):
                            nc.tensor.transpose(
                                out=p2n[:, 128 * hc:128 * hc + 128],
                                in_=nstage[:, 128 * hc:128 * hc + 128],
                                identity=ident_bf[:, :],
                            )
                        for hc in range(4):
                            nc.vector.tensor_scalar(
                                out=gxnT[:, 2048 * hc + 128 * j:2048 * hc + 128 * j + 128],
                                in0=p2n[:, 128 * hc:128 * hc + 128],
                                scalar1=bias_nT[:, hc:hc + 1], scalar2=None, op0=ADD,
                            )

            # ---------- pred task queue (flipped: psum rows = vocab) ----------
            pred_tasks = []     # (bto, n, vc)
            pend_evicts = []    # (pp, bto, n, vc)

            def emit_pred_mms():
                bto, n, vc = pred_tasks.pop(0)
                pv = 128 if vc < 31 else VS - 31 * 128      # last chunk is 80 rows
                pp = psp.tile([128, 512], f32, tag="pp")
                for k in range(4):
                    nc.tensor.matmul(
                        out=pp[0:pv, 0:n],
                        lhsT=predwT[:, (4 * vc + k) * 128:(4 * vc + k) * 128 + pv],
                        rhs=outT[:, 2048 * k + bto:2048 * k + bto + n],
                        start=(k == 0), stop=(k == 3),
                    )
                pend_evicts.append((pp, bto, n, vc))

            def flush_pred_evicts():
                while pend_evicts:
                    pp, bto, n, vc = pend_evicts.pop(0)
                    pv = 128 if vc < 31 else VS - 31 * 128
                    stage = sb3.tile([128, 512], bf16, tag="ostage")
                    nc.vector.tensor_scalar(
                        out=stage[0:pv, 0:n], in0=pp[0:pv, 0:n],
                        scalar1=predbT[0:pv, vc:vc + 1], scalar2=None, op0=ADD,
                    )
                    nc.sync.dma_start(OUT[128 * vc:128 * vc + pv, bto:bto + n],
                                      stage[0:pv, 0:n])

            # ---------- one GRU step ----------
            def emit_step(t, fillers=0):
                jj = t % 8
                w = jj // 2
                par = jj % 2
                tj = t // 8
                cur = t % 2
                prv = 1 - cur
                sel_h = sel[32 * w:32 * w + 32, 32 * par:32 * par + 32]
                sel_h0 = sel[0:32, 32 * par:32 * par + 32]

                pg = psg.tile([128, 512], f32, tag="pg")
                # psum preloads: gx for z,r strips; bn (x128) for n strip
                for grp in range(2):
                    nc.tensor.matmul(
                        out=pg[32 * grp:32 * grp + 32, :],
                        lhsT=sel_h,
                        rhs=gx_rz[32 * w:32 * w + 32,
                                  1024 * tj + 512 * grp:1024 * tj + 512 * grp + 512],
                        start=True, stop=False, skip_group_check=True,
                        tile_position=(32 * w, 32 * grp),
                    )
                nc.tensor.matmul(
                    out=pg[64:96, :], lhsT=sel_h0, rhs=bnb[:, :],
                    start=True, stop=False, skip_group_check=True,
                    tile_position=(0, 64),
                )
                # recurrent matmuls, col-tiled across the 3 gate groups
                if t == 0:
                    st_bf = h0bf
                    sofs = 0
                else:
                    st_bf = outT
                    sofs = 16 * (t - 1)
                for k in range(4):
                    kofs = (2048 * k + sofs) if t > 0 else (16 * k)
                    for grp in range(3):
                        nc.tensor.matmul(
                            out=pg[32 * grp:32 * grp + 32, :],
                            lhsT=st_bf[:, kofs:kofs + 32],
                            rhs=whhT[:, 1536 * k + 512 * grp:1536 * k + 512 * grp + 512],
                            start=False, stop=(k == 3), skip_group_check=True,
                            tile_position=(0, 32 * grp),
                        )
                # fillers slot into the sigmoid window on PE
                for _ in range(fillers):
                    if pred_tasks:
                        emit_pred_mms()
                # sigmoid z,r on ACT; n-strip eviction on DVE in parallel
                # (separate tiles so the two writes don't serialize)
                gz = sb3.tile([128, 512], bf16, tag="gz")
                gn = sb3.tile([128, 512], bf16, tag="gn")
                nc.scalar.activation(gz[0:48, :], pg[0:48, :], SIG, scale=1.0 / GSC)
                nc.vector.tensor_copy(gn[32:48, :], pg[64:80, :])
                # mb = r * (h@Wn + bn) in B-layout; SB inputs share base partition 32
                mbB = sb3.tile([128, 512], bf16, tag="mbB")
                nc.vector.tensor_tensor(out=mbB[32:48, :], in0=gz[32:48, :],
                                        in1=gn[32:48, :], op=MUL)
                # transpose to H-layout: z,r windows then mb windows
                p2 = pst.tile([128, 256], bf16, tag="p2")
                for hc in range(4):
                    nc.tensor.transpose(
                        out=p2[:, 48 * hc:48 * hc + 48],
                        in_=gz[0:48, 128 * hc:128 * hc + 128],
                        identity=ident_bf[0:48, 0:48],
                    )
                for hc in range(4):
                    nc.tensor.transpose(
                        out=p2[:, 192 + 16 * hc:192 + 16 * hc + 16],
                        in_=mbB[32:48, 128 * hc:128 * hc + 128],
                        identity=ident_bf[32:48, 32:48],
                    )
                p2v = p2[:, 0:192].rearrange("p (hc c) -> p hc c", hc=4)
                zT = p2v[:, :, 0:16]
                mbT = p2[:, 192:256].rearrange("p (hc c) -> p hc c", hc=4)
                nin = sb3.tile([128, 64], bf16, tag="nin")
                gslice = gxnT[:, :].rearrange("p (hc c) -> p hc c", hc=4)[
                    :, :, 16 * t:16 * t + 16]
                nc.vector.tensor_tensor(
                    out=nin[:, :].rearrange("p (hc c) -> p hc c", hc=4),
                    in0=mbT, in1=gslice, op=ADD)
                nT = sb3.tile([128, 64], f32, tag="nT")
                nc.scalar.activation(nT[:, :], nin[:, :], TANH, scale=1.0 / GSC)
                # off-chain: zp = 1-z, zh = z*h_prev
                zp = sb3.tile([128, 64], f32, tag="zp")
                nc.vector.tensor_scalar(
                    out=zp[:, :].rearrange("p (hc c) -> p hc c", hc=4),
                    in0=zT, scalar1=-1.0, scalar2=1.0, op0=MUL, op1=ADD)
                zh = sb3.tile([128, 64], f32, tag="zh")
                nc.vector.tensor_tensor(
                    out=zh[:, :].rearrange("p (hc c) -> p hc c", hc=4), in0=zT,
                    in1=state[:, 64 * prv:64 * prv + 64].rearrange(
                        "p (hc c) -> p hc c", hc=4), op=MUL)
                # chain: t2 = (1-z)*n ; h_new = t2 + zh (bf16 write feeds next mm)
                t2 = sb3.tile([128, 64], f32, tag="t2")
                nc.vector.tensor_tensor(out=t2[:, :], in0=zp[:, :], in1=nT[:, :], op=MUL)
                oslice = outT[:, 0:4 * 2048].rearrange("p (hc c) -> p hc c", hc=4)[
                    :, :, 16 * t:16 * t + 16]
                t2v = t2[:, :].rearrange("p (hc c) -> p hc c", hc=4)
                zhv = zh[:, :].rearrange("p (hc c) -> p hc c", hc=4)
                nc.vector.tensor_tensor(out=oslice, in0=t2v, in1=zhv, op=ADD)
                st_new = state[:, 64 * cur:64 * cur + 64]
                nc.vector.tensor_tensor(
                    out=st_new.rearrange("p (hc c) -> p hc c", hc=4),
                    in0=t2v, in1=zhv, op=ADD)
                # pred evictions queue on DVE behind the chain ops
                flush_pred_evicts()

            # ---------- schedule ----------
            emit_gx_tile(0)
            emit_gx_tile(1)
            chunk_i = 0
            for t in range(S):
                if t % 8 == 0 and t // 8 + 2 < NT:
                    emit_gx_tile(t // 8 + 2)
                while chunk_i < len(PRED_CHUNKS):
                    bto, n = PRED_CHUNKS[chunk_i]
                    if 16 * t >= bto + n:
                        pred_tasks.extend((bto, n, vc) for vc in range(32))
                        chunk_i += 1
                    else:
                        break
                rate = 1 if t < 96 else 2
                emit_step(t, fillers=rate)
            # post-scan: release remaining chunks and drain
            while chunk_i < len(PRED_CHUNKS):
                bto, n = PRED_CHUNKS[chunk_i]
                pred_tasks.extend((bto, n, vc) for vc in range(32))
                chunk_i += 1
            while pred_tasks:
                emit_pred_mms()
                flush_pred_evicts()

    nc.compile()
    _PROG_CACHE["nc"] = nc
    return nc


def prep_inputs(y, hn, emb, W_ih, W_hh, b_ih, b_hh, fc_w, fc_b, pred_w, pred_b):
    """Host-side layout prep. Returns per-core in_maps."""
    y = np.asarray(y)
    hn = np.asarray(hn, np.float32)
    emb = np.asarray(emb, np.float32)
    W_ih = np.asarray(W_ih, np.float32)
    W_hh = np.asarray(W_hh, np.float32)
    b_ih = np.asarray(b_ih, np.float32)
    b_hh = np.asarray(b_hh, np.float32)
    fc_w = np.asarray(fc_w, np.float32)
    fc_b = np.asarray(fc_b, np.float32)
    pred_w = np.asarray(pred_w, np.float32)
    pred_b = np.asarray(pred_b, np.float32)

    y_tm = np.ascontiguousarray(y.T.reshape(B * S, 1)).astype(np.int32)

    hn2 = hn[:, 0, :]
    hnT = np.zeros((128, 8 * 16), np.float32)
    for kc in range(8):
        hnT[:, 16 * kc:16 * kc + 16] = hn2[:, 128 * kc:128 * kc + 128].T
    fcwT = np.zeros((128, 8 * 512), np.float32)
    for kc in range(8):
        fcwT[:, 512 * kc:512 * kc + 512] = fc_w[:, 128 * kc:128 * kc + 128].T
    fcbT = np.ascontiguousarray(fc_b.reshape(4, 128).T)

    # gate reorder to z, r, n (reference order r,z,n); W_hh scaled x128
    Wr, Wz, Wn = W_hh[:H], W_hh[H:2 * H], W_hh[2 * H:]
    Wg = np.concatenate([Wz, Wr, Wn], axis=0) * GSC
    whhT = np.zeros((128, 4 * G3), np.float32)
    for k in range(4):
        whhT[:, G3 * k:G3 * k + G3] = Wg[:, 128 * k:128 * k + 128].T
    whhT = whhT.astype(ml_dtypes.bfloat16)

    # W_ih fp8 (DoubleRow pairs): x scaled x32 on device, W_ih x4 -> gx x128
    WIr, WIz, WIn = W_ih[:H], W_ih[H:2 * H], W_ih[2 * H:]
    WIg = np.concatenate([WIz, WIr, WIn], axis=0) * 4.0   # [3H, E]
    # wihT8[p, pair, kt, g, c] = WIg[512*g + c, 128*(2*pair+kt) + p]
    wihT8 = np.zeros((128, 2, 2, 3, 512), np.float32)
    for pr in range(2):
        for kt in range(2):
            ec = 2 * pr + kt
            blk = WIg[:, 128 * ec:128 * ec + 128].T            # [128, 3H]
            wihT8[:, pr, kt, :, :] = blk.reshape(128, 3, 512)
    wihT8 = np.ascontiguousarray(wihT8.reshape(128, -1)).astype(ml_dtypes.float8_e4m3)

    bias_rz = np.zeros((128, 1024), np.float32)
    bias_rz[:, 0:512] = (b_ih[H:2 * H] + b_hh[H:2 * H])[None, :] * GSC   # z
    bias_rz[:, 512:1024] = (b_ih[0:H] + b_hh[0:H])[None, :] * GSC        # r
    bias_rz = bias_rz.astype(ml_dtypes.bfloat16)
    bias_nT = np.ascontiguousarray((b_ih[2 * H:] * GSC).reshape(4, 128).T)
    bnb = np.broadcast_to((b_hh[2 * H:] * GSC)[None, :], (32, 512)).astype(ml_dtypes.bfloat16)
    bnb = np.ascontiguousarray(bnb)

    selmat = np.zeros((32, 64), np.float32)
    for m in range(32):
        selmat[m % 32, m] = 1.0
        selmat[(16 + m) % 32, 32 + m] = 1.0
    selmat = np.tile(selmat, (4, 1)).astype(ml_dtypes.bfloat16)

    in_maps = []
    for c in range(NC):
        pw = np.zeros((4096, H), np.float32)
        pw[:VS] = pred_w[VS * c:VS * c + VS]           # [VS, H] zero-padded
        # predwT[p, (4*vc+k)*128 + v] = pw[128*vc + v, 128*k + p]
        predwT = np.zeros((128, 32, 4, 128), np.float32)
        for vc in range(32):
            blk = pw[128 * vc:128 * vc + 128, :]       # [128 v, 512 h]
            predwT[:, vc, :, :] = blk.T.reshape(4, 128, 128).transpose(1, 0, 2)
        predwT = np.ascontiguousarray(predwT.reshape(128, -1)).astype(ml_dtypes.bfloat16)
        pb = np.zeros((4096,), np.float32)
        pb[:VS] = pred_b[VS * c:VS * c + VS]
        predbT = np.ascontiguousarray(pb.reshape(32, 128).T.astype(np.float32))
        in_maps.append({
            "emb": emb, "y_tm": y_tm, "hnT": hnT, "fcwT": fcwT, "fcbT": fcbT,
            "whhT": whhT, "wihT8": wihT8, "bias_rz": bias_rz, "bias_nT": bias_nT,
            "bnb": bnb, "sel": selmat, "predwT": predwT, "predbT": predbT,
        })
    return in_maps


def kernel(**inputs):
    nc = build_program()
    in_maps = prep_inputs(**inputs)
    res = bass_utils.run_bass_kernel_spmd(nc, in_maps, core_ids=list(range(NC)))
    shards = [np.asarray(res.results[c]["out"], np.float32) for c in range(NC)]
    full = np.concatenate(shards, axis=0)                     # [V, B*S]
    out = full.reshape(V, S, B).transpose(2, 1, 0)            # [B, S, V]
    return np.ascontiguousarray(out)
